# revision 12
# baseline (speedup 1.0000x reference)
"""GCCF (gnn message passing) Bass kernel for 8 trn2 NeuronCores.

Model (reference.py):
  3 layers of bipartite graph propagation:
    u_l = LReLU((user_adj @ m_{l-1} + u_{l-1}) @ Wu[l].T + 2*bu[l])
    m_l = LReLU((movie_adj @ u_{l-1} + m_{l-1}) @ Wm[l].T + 2*bm[l])
  then 100k (uid, mid) pair interactions:
    out[b] = sum_l (u_l[uid] * m_l[mid]) . wo_l + bo

Distribution (8 cores):
  - adjacency rows sharded: core c owns users [2000c, 2000c+2000) and
    movies [1000c, 1000c+1000); each core computes its slice of u_l/m_l
    against the full (all-gathered) opposite-side embedding.
  - embeddings are kept TRANSPOSED on-chip ([E, n]) so both the A@emb
    matmul and the ExE matmul contract on the partition axis with no
    per-layer transposes; adjacency tiles are transposed once (PE
    transpose) in layer 1 and cached to DRAM as bf16 A^T scratch, which
    layers 2..3 stream directly.
  - interaction pairs are bucketed by uid owner on the host; each core
    gathers u-rows from its local (wo-scaled) tables and m-rows from
    all-gathered movie tables via dma_gather, then multiply-reduces.

Precision: adjacency + stationary embeddings in bf16 (error ~1e-4
relative), everything else fp32.
"""
import sys
import threading

sys.path.insert(0, "/opt/trn_rl_repo")

import numpy as np

import concourse.bacc as bacc
import concourse.mybir as mybir
import concourse.tile as tile
from concourse.bass_utils import run_bass_kernel_spmd
from concourse.masks import make_identity

dt = mybir.dt
F32, BF16, I16 = dt.float32, dt.bfloat16, dt.int16
ALU = mybir.AluOpType
AXIS = mybir.AxisListType
ACTF = mybir.ActivationFunctionType

NCORES = 8
NU, NM, E, L, B = 16000, 8000, 64, 3, 100000
UPC, MPC = NU // NCORES, NM // NCORES        # rows per core: 2000 users, 1000 movies
UP, MP = 2048, 1024                          # padded to multiples of 512
KU = [(k, min(128, NM - k * 128)) for k in range((NM + 127) // 128)]   # 63 movie k-tiles
KM = [(k, min(128, NU - k * 128)) for k in range((NU + 127) // 128)]   # 125 user k-tiles
NUCH, NMCH = UP // 512, MP // 512            # output chunks per side (4, 2)
CHUNK = 1024                                 # pairs per dma_gather (>=2048 wedges the DGE ring)
NCH = 14                                     # chunks per core
SC = CHUNK // 128                            # result slots per chunk (8)
CAP = CHUNK * NCH                            # padded pairs per core (14336)


def _emit(nc, tc, io):
    ctxs = []

    def pool(*a, **kw):
        p = tc.tile_pool(*a, **kw)
        ctxs.append(p)
        return p.__enter__()

    const = pool(name="const", bufs=1)
    ident_bf = const.tile([128, 128], BF16)
    make_identity(nc, ident_bf)
    ident_f32 = const.tile([128, 128], F32)
    make_identity(nc, ident_f32)

    # small constants: Wu^T/Wm^T per layer, biases, wo scales
    wut_sb, wmt_sb, bu2_sb, bm2_sb, wo4_sb = [], [], [], [], []
    for l in range(L):
        w = const.tile([64, 64], F32, tag=f"wut{l}")
        nc.sync.dma_start(out=w[:], in_=io["wut"].ap()[l])
        wut_sb.append(w)
        w = const.tile([64, 64], F32, tag=f"wmt{l}")
        nc.sync.dma_start(out=w[:], in_=io["wmt"].ap()[l])
        wmt_sb.append(w)
        bb = const.tile([64, 1], F32, tag=f"bu2{l}")
        nc.sync.dma_start(out=bb[:], in_=io["bu2"].ap()[l])
        bu2_sb.append(bb)
        bb = const.tile([64, 1], F32, tag=f"bm2{l}")
        nc.sync.dma_start(out=bb[:], in_=io["bm2"].ap()[l])
        bm2_sb.append(bb)
    for l in range(4):
        w = const.tile([64, 1], F32, tag=f"wo{l}")
        nc.sync.dma_start(out=w[:], in_=io["wo4"].ap()[l])
        wo4_sb.append(w)

    res_sb = const.tile([128, NCH * SC], F32)
    const_objs = (ident_bf, ident_f32, wut_sb, wmt_sb, bu2_sb, bm2_sb, wo4_sb, res_sb)

    # ---- DRAM scratch (shared across repeat iterations) ----------------
    auT_d = nc.dram_tensor("auT_d", [NM, UP], BF16, kind="Internal")
    amT_d = nc.dram_tensor("amT_d", [NU, MP], BF16, kind="Internal")
    uhat_d = [
        nc.dram_tensor(f"uhat{l}_d", [UP, 64], F32, kind="Internal")
        for l in range(1, 4)
    ]
    agu_in = [
        nc.dram_tensor(f"agu_in{l}", [UPC, 64], BF16, kind="Internal")
        for l in range(1, 3)
    ]
    agu_out = [
        nc.dram_tensor(f"agu_out{l}", [NU, 64], BF16, kind="Internal")
        for l in range(1, 3)
    ]
    agm_in = [
        nc.dram_tensor(f"agm_in{l}", [MPC, 64], F32, kind="Internal")
        for l in range(1, 4)
    ]
    agm_out = [
        nc.dram_tensor(f"agm_out{l}", [NM, 64], F32, kind="Internal")
        for l in range(1, 4)
    ]

    import os
    _kphase = os.environ.get("KPHASE", "full")
    _nlayers = {"setup": 0, "gather": 0, "l1": 1, "l2": 2, "l3": 3}.get(_kphase, L)
    _krep = int(os.environ.get("KREPEAT", "1"))
    for _it in range(_krep):
        _emit_iter(nc, tc, io, const_objs, _kphase, _nlayers,
                   auT_d, amT_d, uhat_d, agu_in, agu_out, agm_in, agm_out)

    for p in reversed(ctxs):
        p.__exit__(None, None, None)


def _emit_iter(nc, tc, io, const_objs, _kphase, _nlayers,
               auT_d, amT_d, uhat_d, agu_in, agu_out, agm_in, agm_out):
    (ident_bf, ident_f32, wut_sb, wmt_sb, bu2_sb, bm2_sb, wo4_sb, res_sb) = const_objs
    ctxs = []

    def pool(*a, **kw):
        p = tc.tile_pool(*a, **kw)
        ctxs.append(p)
        return p.__enter__()

    # ---- phase-A pools -------------------------------------------------
    natp = pool(name="nat", bufs=4)
    cstp = pool(name="cst", bufs=4)
    autp = pool(name="auT", bufs=6)
    amtp = pool(name="amT", bufs=6)
    ustatp = pool(name="ustat", bufs=2)
    mstatp = pool(name="mstat", bufs=2)
    stgp = pool(name="stg", bufs=3)
    utp = pool(name="uT", bufs=3)
    mtp = pool(name="mT", bufs=3)
    uhtp = pool(name="uhatT", bufs=2)
    ubfp = pool(name="ubf", bufs=2)
    xp = pool(name="x", bufs=3)
    s64p = pool(name="s64", bufs=6)
    accp = pool(name="acc", bufs=4, space="PSUM")
    tpp = pool(name="tp", bufs=2, space="PSUM")
    ps2p = pool(name="ps2", bufs=2, space="PSUM")

    def cast_stat(src_ap, n_rows, statp, sdt):
        """DRAM [n_rows, 64] f32 -> SBUF [128, nt, 64] bf16 stationary."""
        full, rem = n_rows // 128, n_rows % 128
        nt = full + (1 if rem else 0)
        st = statp.tile([128, nt, 64], BF16, tag="stat")
        src3 = src_ap[: full * 128].rearrange("(a p) e -> p a e", p=128)
        CHK = 16
        for s in range(0, full, CHK):
            w = min(CHK, full - s)
            stg = stgp.tile([128, CHK, 64], sdt, tag="stg")
            nc.sync.dma_start(out=stg[:, :w, :], in_=src3[:, s : s + w, :])
            nc.gpsimd.tensor_copy(out=st[:, s : s + w, :], in_=stg[:, :w, :])
        if rem:
            stg = stgp.tile([128, CHK, 64], sdt, tag="stg")
            nc.sync.dma_start(out=stg[:rem, 0, :], in_=src_ap[full * 128 :])
            nc.gpsimd.tensor_copy(out=st[:rem, full, :], in_=stg[:rem, 0, :])
        return st

    def load_stat_bf16(src_ap):
        """DRAM [16000, 64] bf16 -> SBUF [128, 125, 64] bf16, one DMA."""
        st = ustatp.tile([128, 125, 64], BF16, tag="stat")
        nc.sync.dma_start(out=st[:], in_=src_ap.rearrange("(a p) e -> p a e", p=128))
        return st

    def kslice(stat, k, kw):
        return stat[0:kw, k, :]

    # ---- matmul-1: pass 1 (transpose + scratch write + matmul) ---------
    def pass1_side(adj_in, scr_d, n_rows_p, n_cols, stat, kt, psums, tpool, tag):
        """adj natural [n_rows_p, n_cols] f32 -> scratch [n_cols, n_rows_p] bf16,
        accumulating psums[n] [64, 512] = (adj @ stat-emb)^T chunks."""
        nib = n_rows_p // 128                       # natural row blocks
        ngr = (n_cols + 511) // 512                 # 512-col groups
        nk = len(kt)
        for g in range(ngr):
            gw = min(512, n_cols - g * 512)
            njs = (gw + 127) // 128
            t_tiles = [tpool.tile([128, n_rows_p], BF16, tag=tag, name=f"{tag}{j}") for j in range(njs)]
            for i in range(nib):
                nat = natp.tile([128, 512], F32, tag="nat")
                nc.sync.dma_start(
                    out=nat[:, :gw],
                    in_=adj_in.ap()[i * 128 : (i + 1) * 128, g * 512 : g * 512 + gw],
                )
                cst = cstp.tile([128, 512], BF16, tag="cst")
                nc.gpsimd.tensor_copy(out=cst[:, :gw], in_=nat[:, :gw])
                for j in range(njs):
                    jw = min(128, gw - j * 128)
                    tp = tpp.tile([128, 128], BF16, tag="tp")
                    nc.tensor.transpose(
                        tp[:jw, :], cst[:, j * 128 : j * 128 + jw], ident_bf[:]
                    )
                    nc.vector.tensor_copy(
                        out=t_tiles[j][:jw, i * 128 : (i + 1) * 128], in_=tp[:jw, :]
                    )
            for j in range(njs):
                k = g * 4 + j
                kw = kt[k][1]
                for n, ps in enumerate(psums):
                    nc.tensor.matmul(
                        ps[:],
                        kslice(stat, k, kw),
                        t_tiles[j][:kw, n * 512 : (n + 1) * 512],
                        start=(k == 0),
                        stop=(k == nk - 1),
                    )
                nc.sync.dma_start(
                    out=scr_d.ap()[k * 128 : k * 128 + kw, :], in_=t_tiles[j][:kw, :]
                )

    # ---- matmul-1: passes 2..3 (stream scratch) ------------------------
    def passN_side(scr_d, n_rows_p, stat, kt, psums, tpool, tag):
        nk = len(kt)
        for k, kw in kt:
            rt = tpool.tile([128, n_rows_p], BF16, tag=tag)
            nc.sync.dma_start(
                out=rt[:kw, :], in_=scr_d.ap()[k * 128 : k * 128 + kw, :]
            )
            for n, ps in enumerate(psums):
                nc.tensor.matmul(
                    ps[:],
                    kslice(stat, k, kw),
                    rt[:kw, n * 512 : (n + 1) * 512],
                    start=(k == 0),
                    stop=(k == nk - 1),
                )

    # ---- epilogue: x = psum + prevT; x @ W^T; LReLU --------------------
    def epilogue(psums, prevT, w_sb, b_sb, outp, width, tag):
        curT = outp.tile([64, width], F32, tag=tag)
        for n, ps in enumerate(psums):
            x = xp.tile([64, 512], F32, tag="x")
            nc.vector.tensor_tensor(
                x[:], ps[:], prevT[:, n * 512 : (n + 1) * 512], ALU.add
            )
            ps2 = ps2p.tile([64, 512], F32, tag="ps2")
            nc.tensor.matmul(ps2[:], w_sb[:], x[:], start=True, stop=True)
            nc.scalar.activation(
                curT[:, n * 512 : (n + 1) * 512],
                ps2[:],
                ACTF.Lrelu,
                bias=b_sb[:],
                alpha=0.01,
            )
        return curT

    def transpose_out(srcT, cols, dst_ap, ident, sdt):
        """[64, >=cols] srcT -> natural [cols, 64] written to dst_ap rows."""
        for i in range(0, cols, 128):
            cw = min(128, cols - i)
            tp = tpp.tile([128, 128], sdt, tag="tp")
            nc.tensor.transpose(tp[:cw, :64], srcT[:, i : i + cw], ident[0:64, 0:64])
            sbt = s64p.tile([128, 64], sdt, tag="s64" + ("b" if sdt == BF16 else "f"))
            nc.vector.tensor_copy(out=sbt[:cw, :], in_=tp[:cw, :64])
            nc.sync.dma_start(out=dst_ap[i : i + cw, :], in_=sbt[:cw, :])

    def allgather(in_t, out_t):
        import os as _os
        if _os.environ.get("KSIM"):
            # 1-core TimelineSim build: emulate AG cost with local DMAs
            n = in_t.ap().shape[0]
            for r in range(NCORES):
                nc.sync.dma_start(
                    out=out_t.ap()[r * n : (r + 1) * n, :], in_=in_t.ap()
                )
            return
        nc.gpsimd.collective_compute(
            "AllGather",
            ALU.bypass,
            replica_groups=[list(range(NCORES))],
            ins=[in_t.ap().opt()],
            outs=[out_t.ap().opt()],
        )

    # ---- setup: stationaries + transposed embeddings -------------------
    # u_stats[l] / m_stats[l] hold the FULL layer-l embeddings (bf16) used
    # as matmul stationaries by layer l's sides (both sides of layer l
    # consume layer l-1... i.e. index l here is "input to layer l").
    u_stats = {0: cast_stat(io["ue"].ap(), NU, ustatp, F32)}
    m_stats = {0: cast_stat(io["me"].ap(), NM, mstatp, F32)}
    uT = utp.tile([64, UP], F32, tag="uT")
    nc.sync.dma_start(out=uT[:], in_=io["ueT"].ap())
    mT = mtp.tile([64, MP], F32, tag="mT")
    nc.sync.dma_start(out=mT[:], in_=io["meT"].ap())

    # ---- 3 layers ------------------------------------------------------
    for l in range(_nlayers):
        first = l == 0
        # side order: L1=[m,u], L2=[u,m], L3=[m,u]  (zero AG stalls)
        m_first = l != 1

        def do_m(l=l, first=first):
            nonlocal mT
            psums = [accp.tile([64, 512], F32, tag="acc", name=f"psm{i}") for i in range(NMCH)]
            if first:
                pass1_side(io["am"], amT_d, MP, NU, u_stats[l], KM, psums, amtp, "amT")
            else:
                passN_side(amT_d, MP, u_stats[l], KM, psums, amtp, "amT")
            mT = epilogue(psums, mT, wmt_sb[l], bm2_sb[l], mtp, MP, "mT")
            transpose_out(mT, MPC, agm_in[l].ap(), ident_f32, F32)
            allgather(agm_in[l], agm_out[l])
            if l < 2:
                m_stats[l + 1] = cast_stat(agm_out[l].ap(), NM, mstatp, F32)

        def do_u(l=l, first=first):
            nonlocal uT
            psums = [accp.tile([64, 512], F32, tag="acc", name=f"psu{i}") for i in range(NUCH)]
            if first:
                pass1_side(io["au"], auT_d, UP, NM, m_stats[l], KU, psums, autp, "auT")
            else:
                passN_side(auT_d, UP, m_stats[l], KU, psums, autp, "auT")
            uT = epilogue(psums, uT, wut_sb[l], bu2_sb[l], utp, UP, "uT")
            uhatT = uhtp.tile([64, UP], F32, tag="uhatT")
            nc.vector.tensor_scalar_mul(uhatT[:], uT[:], wo4_sb[l + 1][:])
            transpose_out(uhatT, UP, uhat_d[l].ap(), ident_f32, F32)
            if l < 2:
                ubf = ubfp.tile([64, UP], BF16, tag="ubf")
                nc.vector.tensor_copy(out=ubf[:], in_=uT[:])
                transpose_out(ubf, UPC, agu_in[l].ap(), ident_bf, BF16)
                allgather(agu_in[l], agu_out[l])
                u_stats[l + 1] = load_stat_bf16(agu_out[l].ap())

        if m_first:
            do_m()
            do_u()
        else:
            do_u()
            do_m()

    # release phase-A pools
    for p in reversed(ctxs):
        p.__exit__(None, None, None)
    del ctxs[:]

    # ---- interaction gather phase --------------------------------------
    if _kphase != "full" and _kphase != "gather":
        nc.any.memset(res_sb[:], 0.0)
        nc.sync.dma_start(out=io["res"].ap(), in_=res_sb[:])
        for p in reversed(ctxs):
            p.__exit__(None, None, None)
        return
    gp = pool(name="g", bufs=2)
    gi = pool(name="gi", bufs=1)
    NW = CHUNK // 16
    uidx_sb = gi.tile([128, NCH * NW], I16)
    nc.sync.dma_start(out=uidx_sb[:], in_=io["uidx"].ap())
    midx_sb = gi.tile([128, NCH * NW], I16)
    nc.sync.dma_start(out=midx_sb[:], in_=io["midx"].ap())

    tab_u = [io["uhat0"].ap()] + [d.ap() for d in uhat_d]
    tab_m = [io["me"].ap()] + [d.ap() for d in agm_out]

    for ch in range(NCH):
        ug = gp.tile([128, 4 * SC, 64], F32, tag="ug")
        mg = gp.tile([128, 4 * SC, 64], F32, tag="mg")
        for l in range(4):
            nc.gpsimd.dma_gather(
                out_ap=ug[:, l * SC : (l + 1) * SC, :],
                in_ap=tab_u[l],
                idxs_ap=uidx_sb[:, ch * NW : (ch + 1) * NW],
                num_idxs=CHUNK,
                num_idxs_reg=CHUNK,
                elem_size=64,
            )
            nc.gpsimd.dma_gather(
                out_ap=mg[:, l * SC : (l + 1) * SC, :],
                in_ap=tab_m[l],
                idxs_ap=midx_sb[:, ch * NW : (ch + 1) * NW],
                num_idxs=CHUNK,
                num_idxs_reg=CHUNK,
                elem_size=64,
            )
        prod = gp.tile([128, 4 * SC, 64], F32, tag="prod")
        nc.vector.tensor_tensor(prod[:], ug[:], mg[:], ALU.mult)
        red = gp.tile([128, 4 * SC], F32, tag="red")
        nc.vector.tensor_reduce(red[:], prod[:], axis=AXIS.X, op=ALU.add)
        t1 = gp.tile([128, SC], F32, tag="t1")
        t2 = gp.tile([128, SC], F32, tag="t2")
        nc.vector.tensor_tensor(t1[:], red[:, 0 : SC], red[:, SC : 2 * SC], ALU.add)
        nc.vector.tensor_tensor(t2[:], red[:, 2 * SC : 3 * SC], red[:, 3 * SC : 4 * SC], ALU.add)
        nc.vector.tensor_tensor(
            res_sb[:, ch * SC : (ch + 1) * SC], t1[:], t2[:], ALU.add
        )

    nc.sync.dma_start(out=io["res"].ap(), in_=res_sb[:])

    for p in reversed(ctxs):
        p.__exit__(None, None, None)


def _build():
    import os as _os
    ndev = 1 if _os.environ.get("KSIM") else NCORES
    nc = bacc.Bacc("TRN2", num_devices=ndev, debug=False)
    io = {}
    io["au"] = nc.dram_tensor("au", [UP, NM], F32, kind="ExternalInput")
    io["am"] = nc.dram_tensor("am", [MP, NU], F32, kind="ExternalInput")
    io["ue"] = nc.dram_tensor("ue", [NU, E], F32, kind="ExternalInput")
    io["me"] = nc.dram_tensor("me", [NM, E], F32, kind="ExternalInput")
    io["ueT"] = nc.dram_tensor("ueT", [E, UP], F32, kind="ExternalInput")
    io["meT"] = nc.dram_tensor("meT", [E, MP], F32, kind="ExternalInput")
    io["uhat0"] = nc.dram_tensor("uhat0", [UP, E], F32, kind="ExternalInput")
    io["wut"] = nc.dram_tensor("wut", [L, E, E], F32, kind="ExternalInput")
    io["wmt"] = nc.dram_tensor("wmt", [L, E, E], F32, kind="ExternalInput")
    io["bu2"] = nc.dram_tensor("bu2", [L, E, 1], F32, kind="ExternalInput")
    io["bm2"] = nc.dram_tensor("bm2", [L, E, 1], F32, kind="ExternalInput")
    io["wo4"] = nc.dram_tensor("wo4", [4, E, 1], F32, kind="ExternalInput")
    io["uidx"] = nc.dram_tensor("uidx", [128, NCH * (CHUNK // 16)], I16, kind="ExternalInput")
    io["midx"] = nc.dram_tensor("midx", [128, NCH * (CHUNK // 16)], I16, kind="ExternalInput")
    io["res"] = nc.dram_tensor("res", [128, NCH * SC], F32, kind="ExternalOutput")

    with tile.TileContext(nc) as tc:
        _emit(nc, tc, io)
    nc.compile()
    return nc


_cache = threading.local()


def _get_nc():
    nc = getattr(_cache, "nc", None)
    if nc is None:
        nc = _build()
        _cache.nc = nc
    return nc


def _wrap_idx(arr):
    """[CAP] int16 -> [128, NCH*(CHUNK//16)] wrapped layout for dma_gather."""
    w = arr.reshape(NCH, CHUNK // 16, 16).transpose(2, 0, 1)   # [16, NCH, NW]
    w = np.tile(w, (8, 1, 1)).reshape(128, NCH * (CHUNK // 16))
    return np.ascontiguousarray(w)


def _prep_in_maps(user_adj, movie_adj, user_emb, movie_emb, Wu, bu, Wm, bm,
                  Wo, bo, user_id, movie_id):
    user_adj = np.asarray(user_adj, np.float32)
    movie_adj = np.asarray(movie_adj, np.float32)
    user_emb = np.asarray(user_emb, np.float32)
    movie_emb = np.asarray(movie_emb, np.float32)
    Wu, bu = np.asarray(Wu, np.float32), np.asarray(bu, np.float32)
    Wm, bm = np.asarray(Wm, np.float32), np.asarray(bm, np.float32)
    Wo, bo = np.asarray(Wo, np.float32), np.asarray(bo, np.float32)
    user_id = np.asarray(user_id, np.int32)
    movie_id = np.asarray(movie_id, np.int32)

    wo = Wo[0]                                            # [(L+1)*E]
    wut = np.ascontiguousarray(Wu.transpose(0, 2, 1))
    wmt = np.ascontiguousarray(Wm.transpose(0, 2, 1))
    bu2 = np.ascontiguousarray((2.0 * bu).reshape(L, E, 1))
    bm2 = np.ascontiguousarray((2.0 * bm).reshape(L, E, 1))
    wo4 = np.ascontiguousarray(wo.reshape(4, E, 1))

    # bucket pairs by uid owner
    own = user_id // UPC
    order = np.argsort(own, kind="stable")
    counts = np.bincount(own, minlength=NCORES)
    assert counts.max() <= CAP, f"bucket overflow: {counts.max()} > {CAP}"
    starts = np.zeros(NCORES + 1, np.int64)
    np.cumsum(counts, out=starts[1:])

    in_maps = []
    metas = []
    for c in range(NCORES):
        idx_c = order[starts[c] : starts[c + 1]]
        n_c = len(idx_c)
        uid_re = np.zeros(CAP, np.int16)
        mid_c = np.zeros(CAP, np.int16)
        uid_re[:n_c] = (user_id[idx_c] - c * UPC).astype(np.int16)
        mid_c[:n_c] = movie_id[idx_c].astype(np.int16)

        au = np.zeros((UP, NM), np.float32)
        au[:UPC] = user_adj[c * UPC : (c + 1) * UPC]
        am = np.zeros((MP, NU), np.float32)
        am[:MPC] = movie_adj[c * MPC : (c + 1) * MPC]
        ue_sl = np.zeros((UP, E), np.float32)
        ue_sl[:UPC] = user_emb[c * UPC : (c + 1) * UPC]
        me_sl = np.zeros((MP, E), np.float32)
        me_sl[:MPC] = movie_emb[c * MPC : (c + 1) * MPC]

        in_maps.append({
            "au": au,
            "am": am,
            "ue": user_emb,
            "me": movie_emb,
            "ueT": np.ascontiguousarray(ue_sl.T),
            "meT": np.ascontiguousarray(me_sl.T),
            "uhat0": np.ascontiguousarray(ue_sl * wo[:E][None, :]),
            "wut": wut,
            "wmt": wmt,
            "bu2": bu2,
            "bm2": bm2,
            "wo4": wo4,
            "uidx": _wrap_idx(uid_re),
            "midx": _wrap_idx(mid_c),
        })
        metas.append((idx_c, n_c))

    return in_maps, metas, float(bo[0])


def _postprocess(results, metas, bo0):
    out = np.zeros(B, np.float32)
    for c in range(NCORES):
        idx_c, n_c = metas[c]
        r = results[c]["res"]                             # [128, NCH*SC]
        vals = r.reshape(128, NCH, SC).transpose(1, 2, 0).reshape(CAP)
        out[idx_c] = vals[:n_c]
    return out + np.float32(bo0)


def kernel(user_adj, movie_adj, user_emb, movie_emb, Wu, bu, Wm, bm, Wo, bo,
           user_id, movie_id):
    in_maps, metas, bo0 = _prep_in_maps(
        user_adj, movie_adj, user_emb, movie_emb, Wu, bu, Wm, bm, Wo, bo,
        user_id, movie_id,
    )
    nc = _get_nc()
    res = run_bass_kernel_spmd(nc, in_maps, core_ids=list(range(NCORES)))
    return _postprocess(res.results, metas, bo0)


# revision 13
# speedup vs baseline: 45.5275x; 45.5275x over previous
"""GCCF (gnn message passing) Bass kernel for 8 trn2 NeuronCores.

Model (reference.py):
  3 layers of bipartite graph propagation:
    u_l = LReLU((user_adj @ m_{l-1} + u_{l-1}) @ Wu[l].T + 2*bu[l])
    m_l = LReLU((movie_adj @ u_{l-1} + m_{l-1}) @ Wm[l].T + 2*bm[l])
  then 100k (uid, mid) pair interactions:
    out[b] = sum_l (u_l[uid] * m_l[mid]) . wo_l + bo

Distribution (8 cores):
  - adjacency rows sharded: core c owns users [2000c, 2000c+2000) and
    movies [1000c, 1000c+1000); each core computes its slice of u_l/m_l
    against the full (all-gathered) opposite-side embedding.
  - embeddings are kept TRANSPOSED on-chip ([E, n]) so both the A@emb
    matmul and the ExE matmul contract on the partition axis with no
    per-layer transposes; adjacency tiles are transposed once (PE
    transpose) in layer 1 and cached to DRAM as bf16 A^T scratch, which
    layers 2..3 stream directly.
  - interaction pairs are bucketed by uid owner on the host; each core
    gathers u-rows from its local (wo-scaled) tables and m-rows from
    all-gathered movie tables via dma_gather, then multiply-reduces.

Precision: adjacency + stationary embeddings in bf16 (error ~1e-4
relative), everything else fp32.
"""
import sys
import threading

sys.path.insert(0, "/opt/trn_rl_repo")

import numpy as np

import concourse.bacc as bacc
import concourse.mybir as mybir
import concourse.tile as tile
from concourse.bass_utils import run_bass_kernel_spmd
from concourse.masks import make_identity

dt = mybir.dt
F32, BF16, I16 = dt.float32, dt.bfloat16, dt.int16
ALU = mybir.AluOpType
AXIS = mybir.AxisListType
ACTF = mybir.ActivationFunctionType

NCORES = 8
NU, NM, E, L, B = 16000, 8000, 64, 3, 100000
UPC, MPC = NU // NCORES, NM // NCORES        # rows per core: 2000 users, 1000 movies
UP, MP = 2048, 1024                          # padded to multiples of 512
KU = [(k, min(128, NM - k * 128)) for k in range((NM + 127) // 128)]   # 63 movie k-tiles
KM = [(k, min(128, NU - k * 128)) for k in range((NU + 127) // 128)]   # 125 user k-tiles
NUCH, NMCH = UP // 512, MP // 512            # output chunks per side (4, 2)
CHUNK = 1024                                 # pairs per dma_gather (>=2048 wedges the DGE ring)
NCH = 14                                     # chunks per core
SC = CHUNK // 128                            # result slots per chunk (8)
CAP = CHUNK * NCH                            # padded pairs per core (14336)


def _emit(nc, tc, io):
    ctxs = []

    def pool(*a, **kw):
        p = tc.tile_pool(*a, **kw)
        ctxs.append(p)
        return p.__enter__()

    const = pool(name="const", bufs=1)
    ident_bf = const.tile([128, 128], BF16)
    make_identity(nc, ident_bf)
    ident_f32 = const.tile([128, 128], F32)
    make_identity(nc, ident_f32)

    # small constants: Wu^T/Wm^T per layer, biases, wo scales
    wut_sb, wmt_sb, bu2_sb, bm2_sb, wo4_sb = [], [], [], [], []
    for l in range(L):
        w = const.tile([64, 64], F32, tag=f"wut{l}")
        nc.sync.dma_start(out=w[:], in_=io["wut"].ap()[l])
        wut_sb.append(w)
        w = const.tile([64, 64], F32, tag=f"wmt{l}")
        nc.sync.dma_start(out=w[:], in_=io["wmt"].ap()[l])
        wmt_sb.append(w)
        bb = const.tile([64, 1], F32, tag=f"bu2{l}")
        nc.sync.dma_start(out=bb[:], in_=io["bu2"].ap()[l])
        bu2_sb.append(bb)
        bb = const.tile([64, 1], F32, tag=f"bm2{l}")
        nc.sync.dma_start(out=bb[:], in_=io["bm2"].ap()[l])
        bm2_sb.append(bb)
    for l in range(4):
        w = const.tile([64, 1], F32, tag=f"wo{l}")
        nc.sync.dma_start(out=w[:], in_=io["wo4"].ap()[l])
        wo4_sb.append(w)

    res_sb = const.tile([128, NCH * SC], F32)
    const_objs = (ident_bf, ident_f32, wut_sb, wmt_sb, bu2_sb, bm2_sb, wo4_sb, res_sb)

    # ---- DRAM scratch (shared across repeat iterations) ----------------
    auT_d = nc.dram_tensor("auT_d", [NM, UP], BF16, kind="Internal")
    amT_d = nc.dram_tensor("amT_d", [NU, MP], BF16, kind="Internal")
    uhat_d = [
        nc.dram_tensor(f"uhat{l}_d", [UP, 64], F32, kind="Internal")
        for l in range(1, 4)
    ]
    agu_in = [
        nc.dram_tensor(f"agu_in{l}", [UPC, 64], BF16, kind="Internal")
        for l in range(1, 3)
    ]
    agu_out = [
        nc.dram_tensor(f"agu_out{l}", [NU, 64], BF16, kind="Internal")
        for l in range(1, 3)
    ]
    agm_in = [
        nc.dram_tensor(f"agm_in{l}", [MPC, 64], F32, kind="Internal")
        for l in range(1, 4)
    ]
    agm_out = [
        nc.dram_tensor(f"agm_out{l}", [NM, 64], F32, kind="Internal")
        for l in range(1, 4)
    ]

    import os
    _kphase = os.environ.get("KPHASE", "full")
    _nlayers = {"setup": 0, "gather": 0, "l1": 1, "l2": 2, "l3": 3}.get(_kphase, L)
    _krep = int(os.environ.get("KREPEAT", "1"))
    for _it in range(_krep):
        _emit_iter(nc, tc, io, const_objs, _kphase, _nlayers,
                   auT_d, amT_d, uhat_d, agu_in, agu_out, agm_in, agm_out)

    for p in reversed(ctxs):
        p.__exit__(None, None, None)


def _emit_iter(nc, tc, io, const_objs, _kphase, _nlayers,
               auT_d, amT_d, uhat_d, agu_in, agu_out, agm_in, agm_out):
    (ident_bf, ident_f32, wut_sb, wmt_sb, bu2_sb, bm2_sb, wo4_sb, res_sb) = const_objs
    ctxs = []

    def pool(*a, **kw):
        p = tc.tile_pool(*a, **kw)
        ctxs.append(p)
        return p.__enter__()

    # ---- phase-A pools -------------------------------------------------
    natp = pool(name="nat", bufs=4)
    cstp = pool(name="cst", bufs=4)
    autp = pool(name="auT", bufs=6)
    amtp = pool(name="amT", bufs=6)
    ustatp = pool(name="ustat", bufs=2)
    mstatp = pool(name="mstat", bufs=2)
    stgp = pool(name="stg", bufs=3)
    utp = pool(name="uT", bufs=3)
    mtp = pool(name="mT", bufs=3)
    uhtp = pool(name="uhatT", bufs=2)
    ubfp = pool(name="ubf", bufs=2)
    xp = pool(name="x", bufs=3)
    s64p = pool(name="s64", bufs=6)
    accp = pool(name="acc", bufs=4, space="PSUM")
    tpp = pool(name="tp", bufs=2, space="PSUM")
    ps2p = pool(name="ps2", bufs=2, space="PSUM")

    def cast_stat(src_ap, n_rows, statp, sdt):
        """DRAM [n_rows, 64] f32 -> SBUF [128, nt, 64] bf16 stationary."""
        full, rem = n_rows // 128, n_rows % 128
        nt = full + (1 if rem else 0)
        st = statp.tile([128, nt, 64], BF16, tag="stat")
        src3 = src_ap[: full * 128].rearrange("(a p) e -> p a e", p=128)
        CHK = 16
        for s in range(0, full, CHK):
            w = min(CHK, full - s)
            stg = stgp.tile([128, CHK, 64], sdt, tag="stg")
            nc.sync.dma_start(out=stg[:, :w, :], in_=src3[:, s : s + w, :])
            nc.gpsimd.tensor_copy(out=st[:, s : s + w, :], in_=stg[:, :w, :])
        if rem:
            stg = stgp.tile([128, CHK, 64], sdt, tag="stg")
            nc.sync.dma_start(out=stg[:rem, 0, :], in_=src_ap[full * 128 :])
            nc.gpsimd.tensor_copy(out=st[:rem, full, :], in_=stg[:rem, 0, :])
        return st

    def load_stat_bf16(src_ap):
        """DRAM [16000, 64] bf16 -> SBUF [128, 125, 64] bf16, one DMA."""
        st = ustatp.tile([128, 125, 64], BF16, tag="stat")
        nc.sync.dma_start(out=st[:], in_=src_ap.rearrange("(a p) e -> p a e", p=128))
        return st

    def kslice(stat, k, kw):
        return stat[0:kw, k, :]

    # ---- matmul-1: pass 1 (transpose + scratch write + matmul) ---------
    def pass1_side(adj_in, scr_d, n_rows_p, n_cols, stat, kt, psums, tpool, tag):
        """adj natural [n_rows_p, n_cols] f32 -> scratch [n_cols, n_rows_p] bf16,
        accumulating psums[n] [64, 512] = (adj @ stat-emb)^T chunks."""
        nib = n_rows_p // 128                       # natural row blocks
        ngr = (n_cols + 511) // 512                 # 512-col groups
        nk = len(kt)
        for g in range(ngr):
            gw = min(512, n_cols - g * 512)
            njs = (gw + 127) // 128
            t_tiles = [tpool.tile([128, n_rows_p], BF16, tag=tag, name=f"{tag}{j}") for j in range(njs)]
            for i in range(nib):
                nat = natp.tile([128, 512], F32, tag="nat")
                nc.sync.dma_start(
                    out=nat[:, :gw],
                    in_=adj_in.ap()[i * 128 : (i + 1) * 128, g * 512 : g * 512 + gw],
                )
                cst = cstp.tile([128, 512], BF16, tag="cst")
                nc.gpsimd.tensor_copy(out=cst[:, :gw], in_=nat[:, :gw])
                for j in range(njs):
                    jw = min(128, gw - j * 128)
                    tp = tpp.tile([128, 128], BF16, tag="tp")
                    nc.tensor.transpose(
                        tp[:jw, :], cst[:, j * 128 : j * 128 + jw], ident_bf[:]
                    )
                    nc.vector.tensor_copy(
                        out=t_tiles[j][:jw, i * 128 : (i + 1) * 128], in_=tp[:jw, :]
                    )
            for j in range(njs):
                k = g * 4 + j
                kw = kt[k][1]
                for n, ps in enumerate(psums):
                    nc.tensor.matmul(
                        ps[:],
                        kslice(stat, k, kw),
                        t_tiles[j][:kw, n * 512 : (n + 1) * 512],
                        start=(k == 0),
                        stop=(k == nk - 1),
                    )
                nc.sync.dma_start(
                    out=scr_d.ap()[k * 128 : k * 128 + kw, :], in_=t_tiles[j][:kw, :]
                )

    # ---- matmul-1: passes 2..3 (stream scratch) ------------------------
    def passN_side(scr_d, n_rows_p, stat, kt, psums, tpool, tag):
        nk = len(kt)
        for k, kw in kt:
            rt = tpool.tile([128, n_rows_p], BF16, tag=tag)
            nc.sync.dma_start(
                out=rt[:kw, :], in_=scr_d.ap()[k * 128 : k * 128 + kw, :]
            )
            for n, ps in enumerate(psums):
                nc.tensor.matmul(
                    ps[:],
                    kslice(stat, k, kw),
                    rt[:kw, n * 512 : (n + 1) * 512],
                    start=(k == 0),
                    stop=(k == nk - 1),
                )

    # ---- epilogue: x = psum + prevT; x @ W^T; LReLU --------------------
    def epilogue(psums, prevT, w_sb, b_sb, outp, width, tag):
        curT = outp.tile([64, width], F32, tag=tag)
        for n, ps in enumerate(psums):
            x = xp.tile([64, 512], F32, tag="x")
            nc.vector.tensor_tensor(
                x[:], ps[:], prevT[:, n * 512 : (n + 1) * 512], ALU.add
            )
            ps2 = ps2p.tile([64, 512], F32, tag="ps2")
            nc.tensor.matmul(ps2[:], w_sb[:], x[:], start=True, stop=True)
            nc.scalar.activation(
                curT[:, n * 512 : (n + 1) * 512],
                ps2[:],
                ACTF.Lrelu,
                bias=b_sb[:],
                alpha=0.01,
            )
        return curT

    def transpose_out(srcT, cols, dst_ap, ident, sdt):
        """[64, >=cols] srcT -> natural [cols, 64] written to dst_ap rows."""
        for i in range(0, cols, 128):
            cw = min(128, cols - i)
            tp = tpp.tile([128, 128], sdt, tag="tp")
            nc.tensor.transpose(tp[:cw, :64], srcT[:, i : i + cw], ident[0:64, 0:64])
            sbt = s64p.tile([128, 64], sdt, tag="s64" + ("b" if sdt == BF16 else "f"))
            nc.vector.tensor_copy(out=sbt[:cw, :], in_=tp[:cw, :64])
            nc.sync.dma_start(out=dst_ap[i : i + cw, :], in_=sbt[:cw, :])

    def allgather(in_t, out_t):
        import os as _os
        if _os.environ.get("KSIM") or _os.environ.get("KNOCC"):
            # timing-only variant: emulate AG with local DMAs (wrong results)
            n = in_t.ap().shape[0]
            for r in range(NCORES):
                nc.sync.dma_start(
                    out=out_t.ap()[r * n : (r + 1) * n, :], in_=in_t.ap()
                )
            return
        nc.gpsimd.collective_compute(
            "AllGather",
            ALU.bypass,
            replica_groups=[list(range(NCORES))],
            ins=[in_t.ap().opt()],
            outs=[out_t.ap().opt()],
        )

    # ---- setup: stationaries + transposed embeddings -------------------
    # u_stats[l] / m_stats[l] hold the FULL layer-l embeddings (bf16) used
    # as matmul stationaries by layer l's sides (both sides of layer l
    # consume layer l-1... i.e. index l here is "input to layer l").
    u_stats = {0: cast_stat(io["ue"].ap(), NU, ustatp, F32)}
    m_stats = {0: cast_stat(io["me"].ap(), NM, mstatp, F32)}
    uT = utp.tile([64, UP], F32, tag="uT")
    nc.sync.dma_start(out=uT[:], in_=io["ueT"].ap())
    mT = mtp.tile([64, MP], F32, tag="mT")
    nc.sync.dma_start(out=mT[:], in_=io["meT"].ap())

    # ---- 3 layers ------------------------------------------------------
    for l in range(_nlayers):
        first = l == 0
        # side order: L1=[m,u], L2=[u,m], L3=[m,u]  (zero AG stalls)
        m_first = l != 1

        def do_m(l=l, first=first):
            nonlocal mT
            psums = [accp.tile([64, 512], F32, tag="acc", name=f"psm{i}") for i in range(NMCH)]
            if first:
                pass1_side(io["am"], amT_d, MP, NU, u_stats[l], KM, psums, amtp, "amT")
            else:
                passN_side(amT_d, MP, u_stats[l], KM, psums, amtp, "amT")
            mT = epilogue(psums, mT, wmt_sb[l], bm2_sb[l], mtp, MP, "mT")
            transpose_out(mT, MPC, agm_in[l].ap(), ident_f32, F32)
            allgather(agm_in[l], agm_out[l])
            if l < 2:
                m_stats[l + 1] = cast_stat(agm_out[l].ap(), NM, mstatp, F32)

        def do_u(l=l, first=first):
            nonlocal uT
            psums = [accp.tile([64, 512], F32, tag="acc", name=f"psu{i}") for i in range(NUCH)]
            if first:
                pass1_side(io["au"], auT_d, UP, NM, m_stats[l], KU, psums, autp, "auT")
            else:
                passN_side(auT_d, UP, m_stats[l], KU, psums, autp, "auT")
            uT = epilogue(psums, uT, wut_sb[l], bu2_sb[l], utp, UP, "uT")
            uhatT = uhtp.tile([64, UP], F32, tag="uhatT")
            nc.vector.tensor_scalar_mul(uhatT[:], uT[:], wo4_sb[l + 1][:])
            transpose_out(uhatT, UP, uhat_d[l].ap(), ident_f32, F32)
            if l < 2:
                ubf = ubfp.tile([64, UP], BF16, tag="ubf")
                nc.vector.tensor_copy(out=ubf[:], in_=uT[:])
                transpose_out(ubf, UPC, agu_in[l].ap(), ident_bf, BF16)
                allgather(agu_in[l], agu_out[l])
                u_stats[l + 1] = load_stat_bf16(agu_out[l].ap())

        if m_first:
            do_m()
            do_u()
        else:
            do_u()
            do_m()

    # release phase-A pools
    for p in reversed(ctxs):
        p.__exit__(None, None, None)
    del ctxs[:]

    # ---- interaction gather phase --------------------------------------
    if _kphase != "full" and _kphase != "gather":
        nc.any.memset(res_sb[:], 0.0)
        nc.sync.dma_start(out=io["res"].ap(), in_=res_sb[:])
        for p in reversed(ctxs):
            p.__exit__(None, None, None)
        return
    gp = pool(name="g", bufs=2)
    gi = pool(name="gi", bufs=1)
    NW = CHUNK // 16
    uidx_sb = gi.tile([128, NCH * NW], I16)
    nc.sync.dma_start(out=uidx_sb[:], in_=io["uidx"].ap())
    midx_sb = gi.tile([128, NCH * NW], I16)
    nc.sync.dma_start(out=midx_sb[:], in_=io["midx"].ap())

    tab_u = [io["uhat0"].ap()] + [d.ap() for d in uhat_d]
    tab_m = [io["me"].ap()] + [d.ap() for d in agm_out]

    for ch in range(NCH):
        ug = gp.tile([128, 4 * SC, 64], F32, tag="ug")
        mg = gp.tile([128, 4 * SC, 64], F32, tag="mg")
        for l in range(4):
            nc.gpsimd.dma_gather(
                out_ap=ug[:, l * SC : (l + 1) * SC, :],
                in_ap=tab_u[l],
                idxs_ap=uidx_sb[:, ch * NW : (ch + 1) * NW],
                num_idxs=CHUNK,
                num_idxs_reg=CHUNK,
                elem_size=64,
            )
            nc.gpsimd.dma_gather(
                out_ap=mg[:, l * SC : (l + 1) * SC, :],
                in_ap=tab_m[l],
                idxs_ap=midx_sb[:, ch * NW : (ch + 1) * NW],
                num_idxs=CHUNK,
                num_idxs_reg=CHUNK,
                elem_size=64,
            )
        prod = gp.tile([128, 4 * SC, 64], F32, tag="prod")
        nc.vector.tensor_tensor(prod[:], ug[:], mg[:], ALU.mult)
        red = gp.tile([128, 4 * SC], F32, tag="red")
        nc.vector.tensor_reduce(red[:], prod[:], axis=AXIS.X, op=ALU.add)
        t1 = gp.tile([128, SC], F32, tag="t1")
        t2 = gp.tile([128, SC], F32, tag="t2")
        nc.vector.tensor_tensor(t1[:], red[:, 0 : SC], red[:, SC : 2 * SC], ALU.add)
        nc.vector.tensor_tensor(t2[:], red[:, 2 * SC : 3 * SC], red[:, 3 * SC : 4 * SC], ALU.add)
        nc.vector.tensor_tensor(
            res_sb[:, ch * SC : (ch + 1) * SC], t1[:], t2[:], ALU.add
        )

    nc.sync.dma_start(out=io["res"].ap(), in_=res_sb[:])

    for p in reversed(ctxs):
        p.__exit__(None, None, None)


def _build():
    import os as _os
    ndev = 1 if _os.environ.get("KSIM") else NCORES
    nc = bacc.Bacc("TRN2", num_devices=ndev, debug=False)
    io = {}
    io["au"] = nc.dram_tensor("au", [UP, NM], F32, kind="ExternalInput")
    io["am"] = nc.dram_tensor("am", [MP, NU], F32, kind="ExternalInput")
    io["ue"] = nc.dram_tensor("ue", [NU, E], F32, kind="ExternalInput")
    io["me"] = nc.dram_tensor("me", [NM, E], F32, kind="ExternalInput")
    io["ueT"] = nc.dram_tensor("ueT", [E, UP], F32, kind="ExternalInput")
    io["meT"] = nc.dram_tensor("meT", [E, MP], F32, kind="ExternalInput")
    io["uhat0"] = nc.dram_tensor("uhat0", [UP, E], F32, kind="ExternalInput")
    io["wut"] = nc.dram_tensor("wut", [L, E, E], F32, kind="ExternalInput")
    io["wmt"] = nc.dram_tensor("wmt", [L, E, E], F32, kind="ExternalInput")
    io["bu2"] = nc.dram_tensor("bu2", [L, E, 1], F32, kind="ExternalInput")
    io["bm2"] = nc.dram_tensor("bm2", [L, E, 1], F32, kind="ExternalInput")
    io["wo4"] = nc.dram_tensor("wo4", [4, E, 1], F32, kind="ExternalInput")
    io["uidx"] = nc.dram_tensor("uidx", [128, NCH * (CHUNK // 16)], I16, kind="ExternalInput")
    io["midx"] = nc.dram_tensor("midx", [128, NCH * (CHUNK // 16)], I16, kind="ExternalInput")
    io["res"] = nc.dram_tensor("res", [128, NCH * SC], F32, kind="ExternalOutput")

    with tile.TileContext(nc) as tc:
        _emit(nc, tc, io)
    nc.compile()
    return nc


_cache = threading.local()


def _get_nc():
    nc = getattr(_cache, "nc", None)
    if nc is None:
        nc = _build()
        _cache.nc = nc
    return nc


def _wrap_idx(arr):
    """[CAP] int16 -> [128, NCH*(CHUNK//16)] wrapped layout for dma_gather."""
    w = arr.reshape(NCH, CHUNK // 16, 16).transpose(2, 0, 1)   # [16, NCH, NW]
    w = np.tile(w, (8, 1, 1)).reshape(128, NCH * (CHUNK // 16))
    return np.ascontiguousarray(w)


def _prep_in_maps(user_adj, movie_adj, user_emb, movie_emb, Wu, bu, Wm, bm,
                  Wo, bo, user_id, movie_id):
    user_adj = np.asarray(user_adj, np.float32)
    movie_adj = np.asarray(movie_adj, np.float32)
    user_emb = np.asarray(user_emb, np.float32)
    movie_emb = np.asarray(movie_emb, np.float32)
    Wu, bu = np.asarray(Wu, np.float32), np.asarray(bu, np.float32)
    Wm, bm = np.asarray(Wm, np.float32), np.asarray(bm, np.float32)
    Wo, bo = np.asarray(Wo, np.float32), np.asarray(bo, np.float32)
    user_id = np.asarray(user_id, np.int32)
    movie_id = np.asarray(movie_id, np.int32)

    wo = Wo[0]                                            # [(L+1)*E]
    wut = np.ascontiguousarray(Wu.transpose(0, 2, 1))
    wmt = np.ascontiguousarray(Wm.transpose(0, 2, 1))
    bu2 = np.ascontiguousarray((2.0 * bu).reshape(L, E, 1))
    bm2 = np.ascontiguousarray((2.0 * bm).reshape(L, E, 1))
    wo4 = np.ascontiguousarray(wo.reshape(4, E, 1))

    # bucket pairs by uid owner
    own = user_id // UPC
    order = np.argsort(own, kind="stable")
    counts = np.bincount(own, minlength=NCORES)
    assert counts.max() <= CAP, f"bucket overflow: {counts.max()} > {CAP}"
    starts = np.zeros(NCORES + 1, np.int64)
    np.cumsum(counts, out=starts[1:])

    in_maps = []
    metas = []
    for c in range(NCORES):
        idx_c = order[starts[c] : starts[c + 1]]
        n_c = len(idx_c)
        uid_re = np.zeros(CAP, np.int16)
        mid_c = np.zeros(CAP, np.int16)
        uid_re[:n_c] = (user_id[idx_c] - c * UPC).astype(np.int16)
        mid_c[:n_c] = movie_id[idx_c].astype(np.int16)

        au = np.zeros((UP, NM), np.float32)
        au[:UPC] = user_adj[c * UPC : (c + 1) * UPC]
        am = np.zeros((MP, NU), np.float32)
        am[:MPC] = movie_adj[c * MPC : (c + 1) * MPC]
        ue_sl = np.zeros((UP, E), np.float32)
        ue_sl[:UPC] = user_emb[c * UPC : (c + 1) * UPC]
        me_sl = np.zeros((MP, E), np.float32)
        me_sl[:MPC] = movie_emb[c * MPC : (c + 1) * MPC]

        in_maps.append({
            "au": au,
            "am": am,
            "ue": user_emb,
            "me": movie_emb,
            "ueT": np.ascontiguousarray(ue_sl.T),
            "meT": np.ascontiguousarray(me_sl.T),
            "uhat0": np.ascontiguousarray(ue_sl * wo[:E][None, :]),
            "wut": wut,
            "wmt": wmt,
            "bu2": bu2,
            "bm2": bm2,
            "wo4": wo4,
            "uidx": _wrap_idx(uid_re),
            "midx": _wrap_idx(mid_c),
        })
        metas.append((idx_c, n_c))

    return in_maps, metas, float(bo[0])


def _postprocess(results, metas, bo0):
    out = np.zeros(B, np.float32)
    for c in range(NCORES):
        idx_c, n_c = metas[c]
        r = results[c]["res"]                             # [128, NCH*SC]
        vals = r.reshape(128, NCH, SC).transpose(1, 2, 0).reshape(CAP)
        out[idx_c] = vals[:n_c]
    return out + np.float32(bo0)


def kernel(user_adj, movie_adj, user_emb, movie_emb, Wu, bu, Wm, bm, Wo, bo,
           user_id, movie_id):
    in_maps, metas, bo0 = _prep_in_maps(
        user_adj, movie_adj, user_emb, movie_emb, Wu, bu, Wm, bm, Wo, bo,
        user_id, movie_id,
    )
    nc = _get_nc()
    res = run_bass_kernel_spmd(nc, in_maps, core_ids=list(range(NCORES)))
    return _postprocess(res.results, metas, bo0)
